# revision 13
# baseline (speedup 1.0000x reference)
"""MLA-style attention (nn_Attention_7868380086611) on 8 TRN2 NeuronCores.

Strategy
--------
The reference "absorbs" the up-projections (k_eff = Wuq_h @ Wuk_h per head,
v_eff = (W_uv.T @ W_o.T) per-head slices), which is ~4x more FLOPs than the
factored form.  By matmul associativity we instead compute standard per-head
q/k (head dim 128) plus the decoupled-RoPE part, and an effective per-head
v~_h = c_kv @ (W_uv.T @ W_o.T)[:, cols_h], so the [T,T] attention matrix only
ever multiplies 128-wide tensors.

Sharding: head-parallel attention (2 of 16 heads per core) on top of
T-sharded down-projections.  Each core computes c_q/c_kv/k_r for its T/8
token slice (transposed layout, contraction dims on partitions), then one
AllGather (~1 MB/rank, bf16) replicates the tiny latents, and each core runs
the full causal attention for its 2 heads, writing its own 256 output
columns.  All inputs are pre-cast/pre-tiled to bf16 on the host; PSUM
accumulation is fp32.

The same SPMD graph runs on all 8 cores; all rank-dependence is carried by
the per-core input slices.
"""

import math
import sys

import numpy as np

sys.path.insert(0, "/opt/trn_rl_repo")

import ml_dtypes  # noqa: E402

from concourse import bacc, bass, masks, mybir  # noqa: E402
from concourse.bass_utils import run_bass_kernel_spmd  # noqa: E402
from concourse.tile import TileContext  # noqa: E402

B, T, C = 1, 2048, 2048
NH, HS = 16, 128
NLQ, NLKV, DHR = 1536, 512, 64
NCORES = 8
HPC = NH // NCORES          # heads per core = 2
TS = T // NCORES            # 256-token shard for down-projections
P = 128
LQ = NLQ // P               # 12 l-chunks
LKV = NLKV // P             # 4
CCH = C // P                # 16 c-chunks
TJ = T // 512               # 4 t-chunks of 512
SC = T // P                 # 16 s-chunks
SCALE = 1.0 / math.sqrt(HS + DHR)
NEG = -1.0e10

BF = mybir.dt.bfloat16
F32 = mybir.dt.float32
Exp = mybir.ActivationFunctionType.Exp
Copy = mybir.ActivationFunctionType.Copy

GROUP = NLQ + NLKV + DHR    # 2112 rows in the all-gather buffer


def build_nc():
    nc = bacc.Bacc(None, target_bir_lowering=False, num_devices=NCORES)

    xT_sh = nc.declare_dram_parameter("xT_sh", [C, TS], BF, isOutput=False)
    wdqT = nc.declare_dram_parameter("wdqT", [LQ // 4, C, 512], BF, isOutput=False)
    wdkvT = nc.declare_dram_parameter("wdkvT", [1, C, 512], BF, isOutput=False)
    wkrT = nc.declare_dram_parameter("wkrT", [C, DHR], BF, isOutput=False)
    cos2T = nc.declare_dram_parameter("cos2T", [DHR, T], BF, isOutput=False)
    sin2T = nc.declare_dram_parameter("sin2T", [DHR, T], BF, isOutput=False)
    wuq = nc.declare_dram_parameter("wuq", [LQ, P, HPC * HS], BF, isOutput=False)
    wqrT = nc.declare_dram_parameter("wqrT", [LQ, P, HPC * DHR], BF, isOutput=False)
    wukT = nc.declare_dram_parameter("wukT", [LKV, P, HPC * HS], BF, isOutput=False)
    wuv = nc.declare_dram_parameter("wuv", [CCH, P, NLKV], BF, isOutput=False)
    woT = nc.declare_dram_parameter("woT", [CCH, P, HPC * HS], BF, isOutput=False)
    out = nc.declare_dram_parameter("out", [T, HPC * HS], F32, isOutput=True)

    GKV = NLKV + DHR
    cc_in_kv = nc.dram_tensor("cc_in_kv", [GKV, TS], BF)
    cc_out_kv = nc.dram_tensor("cc_out_kv", [NCORES, GKV, TS], BF,
                               addr_space="Shared")
    cc_in_q = nc.dram_tensor("cc_in_q", [NLQ, TS], BF)
    cc_out_q = nc.dram_tensor("cc_out_q", [NCORES, NLQ, TS], BF,
                              addr_space="Shared")

    with TileContext(nc) as tc:
        with (
            tc.tile_pool(name="persist", bufs=1) as persist,
            tc.tile_pool(name="lat", bufs=1) as lat,
            tc.tile_pool(name="proj", bufs=1) as proj,
        ):
            # ---- constants ----
            id_bf = persist.tile([P, P], BF)
            masks.make_identity(nc, id_bf[:])
            id_f32 = persist.tile([P, P], F32)
            masks.make_identity(nc, id_f32[:])
            ones_bf = persist.tile([P, 1], BF)
            nc.vector.memset(ones_bf[:], 1.0)
            # 4 additive causal masks [128, 512]: keep (0) iff t - s - 128*m >= 0
            cmask = persist.tile([P, 4 * 512], F32)
            nc.gpsimd.memset(cmask[:], 0.0)
            for m in range(4):
                nc.gpsimd.affine_select(
                    out=cmask[:, m * 512:(m + 1) * 512],
                    in_=cmask[:, m * 512:(m + 1) * 512],
                    compare_op=mybir.AluOpType.is_ge,
                    fill=NEG,
                    base=-m * P,
                    channel_multiplier=-1,
                    pattern=[[1, 512]],
                )
            cos_sb = persist.tile([DHR, T], BF)
            nc.sync.dma_start(cos_sb[:], cos2T[:, :])
            sin_sb = persist.tile([DHR, T], BF)
            nc.sync.dma_start(sin_sb[:], sin2T[:, :])

            # ---- phase 1: local T-shard of c_kv^T/k_r^T (then AG1), c_q^T
            # (then AG2).  Weights stored as [group][C, 512] for 1KB bursts.
            with (
                tc.tile_pool(name="p1w", bufs=2) as p1w,
                tc.tile_pool(name="p1ps", bufs=2, space="PSUM") as p1ps,
                tc.tile_pool(name="p1sh", bufs=3) as p1sh,
            ):
                xt_all = lat.tile([P, CCH * TS], BF)
                nc.sync.dma_start(
                    xt_all[:].rearrange("p (n u) -> p n u", n=CCH),
                    xT_sh.ap().rearrange("(n p) u -> p n u", p=P),
                )

                def down_proj(wparam, group, nsub, bounce, row0):
                    w = p1w.tile([P, CCH * nsub * P], BF, name="p1w_t", tag="p1w_t")
                    nc.sync.dma_start(
                        w[:].rearrange("p (n m) -> p n m", n=CCH),
                        wparam[group].rearrange("(n p) m -> p n m", p=P),
                    )
                    for ls in range(nsub):
                        ps = p1ps.tile([P, TS], F32, name="p1ps_t", tag="p1ps_t")
                        for c in range(CCH):
                            nc.tensor.matmul(
                                ps[:],
                                w[:, c * nsub * P + ls * P: c * nsub * P + (ls + 1) * P],
                                xt_all[:, c * TS:(c + 1) * TS],
                                start=(c == 0),
                                stop=(c == CCH - 1),
                            )
                        sh = p1sh.tile([P, TS], BF, name="p1sh_t", tag="p1sh_t")
                        nc.scalar.copy(sh[:], ps[:])
                        nc.sync.dma_start(
                            bounce[row0 + ls * P: row0 + (ls + 1) * P, :], sh[:]
                        )

                # c_kv (4 l-chunks) then k_r
                down_proj(wdkvT, 0, 4, cc_in_kv, 0)
                wkr_sb = p1w.tile([P, CCH * DHR], BF, name="wkr_sb")
                nc.sync.dma_start(
                    wkr_sb[:].rearrange("p (n m) -> p n m", n=CCH),
                    wkrT.ap().rearrange("(n p) m -> p n m", p=P),
                )
                ps_kr = p1ps.tile([DHR, TS], F32, name="ps_kr", tag="p1ps_t")
                for c in range(CCH):
                    nc.tensor.matmul(
                        ps_kr[:],
                        wkr_sb[:, c * DHR:(c + 1) * DHR],
                        xt_all[:, c * TS:(c + 1) * TS],
                        start=(c == 0),
                        stop=(c == CCH - 1),
                    )
                sh_kr = p1sh.tile([DHR, TS], BF, name="sh_kr")
                nc.scalar.copy(sh_kr[:], ps_kr[:])
                nc.scalar.dma_start(cc_in_kv[NLKV:GKV, :], sh_kr[:])

                nc.gpsimd.collective_compute(
                    "AllGather",
                    mybir.AluOpType.bypass,
                    replica_groups=[list(range(NCORES))],
                    ins=[cc_in_kv.ap().opt()],
                    outs=[cc_out_kv.ap().opt()],
                )

                # c_q (12 l-chunks in 3 groups of 4)
                for g in range(LQ // 4):
                    down_proj(wdqT, g, 4, cc_in_q, g * 4 * P)

            nc.gpsimd.collective_compute(
                "AllGather",
                mybir.AluOpType.bypass,
                replica_groups=[list(range(NCORES))],
                ins=[cc_in_q.ap().opt()],
                outs=[cc_out_q.ap().opt()],
            )

            # ---- B = (W_uv.T @ W_o.T)[:, 2-head cols]  (independent of AG) ----
            b_all = proj.tile([P, LKV * HPC * HS], BF)  # [128, 4*256], m-chunk major
            with (
                tc.tile_pool(name="pbw", bufs=3) as pbw,
                tc.tile_pool(name="pbps", bufs=1, space="PSUM") as pbps,
            ):
                ps_b = [
                    pbps.tile([P, HPC * HS], F32, name=f"ps_b{m}") for m in range(LKV)
                ]
                for c in range(CCH):
                    wuv_t = pbw.tile([P, NLKV], BF, name="wuv_t", tag="wuv_t")
                    nc.sync.dma_start(wuv_t[:], wuv[c])
                    wo_t = pbw.tile([P, HPC * HS], BF, name="wo_t", tag="wo_t")
                    nc.sync.dma_start(wo_t[:], woT[c])
                    for m in range(LKV):
                        nc.tensor.matmul(
                            ps_b[m][:],
                            wuv_t[:, m * P:(m + 1) * P],
                            wo_t[:],
                            start=(c == 0),
                            stop=(c == CCH - 1),
                        )
                for m in range(LKV):
                    nc.scalar.copy(
                        b_all[:, m * HPC * HS:(m + 1) * HPC * HS], ps_b[m][:]
                    )

            # ---- load gathered latents (per-chunk tiles for fine deps) ----
            ckv_t = []
            for l in range(LKV):
                t = lat.tile([P, T], BF, name=f"ckv{l}", tag=f"ckv{l}")
                nc.sync.dma_start(
                    t[:].rearrange("p (g u) -> p g u", g=NCORES),
                    cc_out_kv[:, l * P:(l + 1) * P, :].rearrange("g p u -> p g u"),
                )
                ckv_t.append(t)
            kr_raw = lat.tile([DHR, T], BF)
            nc.sync.dma_start(
                kr_raw[:].rearrange("p (g u) -> p g u", g=NCORES),
                cc_out_kv[:, NLKV:GKV, :].rearrange("g p u -> p g u"),
            )
            cq_t = []
            for l in range(LQ):
                t = lat.tile([P, T], BF, name=f"cq{l}", tag=f"cq{l}")
                nc.sync.dma_start(
                    t[:].rearrange("p (g u) -> p g u", g=NCORES),
                    cc_out_q[:, l * P:(l + 1) * P, :].rearrange("g p u -> p g u"),
                )
                cq_t.append(t)

            # ---- rope on k_r (layout: rows 0-31 = re, 32-63 = im) ----
            kr_rope = proj.tile([DHR, T], BF)
            with tc.tile_pool(name="rtmp", bufs=2) as rtmp:

                def rope(dst, src):
                    # dst = src * [cos;cos] + swap_halves(src) * [-sin;sin]
                    sw = rtmp.tile([DHR, T], BF, name="rsw", tag="rsw")
                    nc.sync.dma_start(sw[0:32, :], src[32:64, :])
                    nc.sync.dma_start(sw[32:64, :], src[0:32, :])
                    ta = rtmp.tile([DHR, T], BF, name="rta", tag="rta")
                    tb = rtmp.tile([DHR, T], BF, name="rtb", tag="rtb")
                    nc.vector.tensor_mul(ta[:], src, cos_sb[:])
                    nc.vector.tensor_mul(tb[:], sw[:], sin_sb[:])
                    nc.vector.tensor_add(dst, ta[:], tb[:])

                rope(kr_rope[:, :], kr_raw[:, :])

                # ---- per-head projections ----
                qT = proj.tile([P, HPC * T], BF)     # q^T   [head][d=128, t]
                kT = proj.tile([P, HPC * T], BF)     # k^T   [head][d=128, s]
                qr_rope = proj.tile([DHR, HPC * T], BF)  # q_r^T [64, head-major t]
                v_sb = proj.tile([P, SC * HPC * HS], BF)  # v~ [s-chunk][s128, 256]

                with (
                    tc.tile_pool(name="p5w", bufs=1) as p5w,
                    tc.tile_pool(name="p5ps", bufs=4, space="PSUM") as p5ps,
                ):
                    wuq_all = p5w.tile([P, LQ * HPC * HS], BF)
                    for l in range(LQ):
                        nc.sync.dma_start(
                            wuq_all[:, l * HPC * HS:(l + 1) * HPC * HS], wuq[l]
                        )
                    wqr_all = p5w.tile([P, LQ * HPC * DHR], BF)
                    for l in range(LQ):
                        nc.sync.dma_start(
                            wqr_all[:, l * HPC * DHR:(l + 1) * HPC * DHR], wqrT[l]
                        )
                    wuk_all = p5w.tile([P, LKV * HPC * HS], BF)
                    for l in range(LKV):
                        nc.sync.dma_start(
                            wuk_all[:, l * HPC * HS:(l + 1) * HPC * HS], wukT[l]
                        )

                    # q^T per head
                    for h in range(HPC):
                        for tj in range(TJ):
                            ps = p5ps.tile([P, 512], F32, name="ps_q", tag="p5")
                            for l in range(LQ):
                                nc.tensor.matmul(
                                    ps[:],
                                    wuq_all[:, l * HPC * HS + h * HS:
                                            l * HPC * HS + (h + 1) * HS],
                                    cq_t[l][:, tj * 512:(tj + 1) * 512],
                                    start=(l == 0),
                                    stop=(l == LQ - 1),
                                )
                            nc.scalar.copy(
                                qT[:, h * T + tj * 512: h * T + (tj + 1) * 512], ps[:]
                            )
                    # q_r^T per head: [64, 512] psum, head on free axis in SBUF
                    qr_raw = proj.tile([DHR, HPC * T], BF)
                    for h in range(HPC):
                        for tj in range(TJ):
                            ps = p5ps.tile([DHR, 512], F32, name="ps_qr", tag="p5")
                            for l in range(LQ):
                                nc.tensor.matmul(
                                    ps[:],
                                    wqr_all[:, l * HPC * DHR + h * DHR:
                                            l * HPC * DHR + (h + 1) * DHR],
                                    cq_t[l][:, tj * 512:(tj + 1) * 512],
                                    start=(l == 0),
                                    stop=(l == LQ - 1),
                                )
                            nc.scalar.copy(
                                qr_raw[:, h * T + tj * 512: h * T + (tj + 1) * 512],
                                ps[:],
                            )
                    for h in range(HPC):
                        rope(qr_rope[:, h * T:(h + 1) * T],
                             qr_raw[:, h * T:(h + 1) * T])

                    # k^T per head
                    for h in range(HPC):
                        for sj in range(TJ):
                            ps = p5ps.tile([P, 512], F32, name="ps_k", tag="p5")
                            for l in range(LKV):
                                nc.tensor.matmul(
                                    ps[:],
                                    wuk_all[:, l * HPC * HS + h * HS:
                                            l * HPC * HS + (h + 1) * HS],
                                    ckv_t[l][:, sj * 512:(sj + 1) * 512],
                                    start=(l == 0),
                                    stop=(l == LKV - 1),
                                )
                            nc.scalar.copy(
                                kT[:, h * T + sj * 512: h * T + (sj + 1) * 512], ps[:]
                            )
                    # v~ per s-chunk: [128 s, 256]
                    for sc in range(SC):
                        ps = p5ps.tile([P, HPC * HS], F32, name="ps_v", tag="p5")
                        for l in range(LKV):
                            nc.tensor.matmul(
                                ps[:],
                                ckv_t[l][:, sc * P:(sc + 1) * P],
                                b_all[:, l * HPC * HS:(l + 1) * HPC * HS],
                                start=(l == 0),
                                stop=(l == LKV - 1),
                            )
                        nc.scalar.copy(
                            v_sb[:, sc * HPC * HS:(sc + 1) * HPC * HS], ps[:]
                        )

                # ---- attention (causal, per head, transposed-scores flow) ----
                with (
                    tc.tile_pool(name="pss", bufs=3, space="PSUM") as pss,
                    tc.tile_pool(name="psy", bufs=2, space="PSUM") as psy,
                    tc.tile_pool(name="psd", bufs=1, space="PSUM") as psd,
                    tc.tile_pool(name="psdt", bufs=1, space="PSUM") as psdt,
                    tc.tile_pool(name="psyt", bufs=1, space="PSUM") as psyt,
                    tc.tile_pool(name="atp", bufs=6) as atp,
                    tc.tile_pool(name="spool", bufs=3) as spool,
                    tc.tile_pool(name="opool", bufs=3) as opool,
                ):
                    for h in range(HPC):
                        for tj in range(TJ):
                            nsc = 4 * (tj + 1)
                            ps_y = psy.tile([P, 512], F32, name="ps_y", tag="psy")
                            ps_d = psd.tile([1, 512], F32, name="ps_d", tag="psd")
                            for k in range(nsc):
                                ps_s = pss.tile([P, 512], F32, name="ps_s", tag="pss")
                                nc.tensor.matmul(
                                    ps_s[:],
                                    kT[:, h * T + k * P: h * T + (k + 1) * P],
                                    qT[:, h * T + tj * 512: h * T + (tj + 1) * 512],
                                    start=True,
                                    stop=False,
                                )
                                nc.tensor.matmul(
                                    ps_s[:],
                                    kr_rope[:, k * P:(k + 1) * P],
                                    qr_rope[:, h * T + tj * 512:
                                            h * T + (tj + 1) * 512],
                                    start=False,
                                    stop=True,
                                )
                                m = k - 4 * tj
                                if m >= 0:
                                    nc.vector.tensor_add(
                                        ps_s[:], ps_s[:],
                                        cmask[:, m * 512:(m + 1) * 512],
                                    )
                                at = atp.tile([P, 512], BF, name="at", tag="at")
                                nc.scalar.activation(at[:], ps_s[:], Exp, scale=SCALE)
                                nc.tensor.matmul(
                                    ps_y[:],
                                    v_sb[:, k * HPC * HS + h * HS:
                                         k * HPC * HS + (h + 1) * HS],
                                    at[:],
                                    start=(k == 0),
                                    stop=(k == nsc - 1),
                                )
                                nc.tensor.matmul(
                                    ps_d[:],
                                    ones_bf[:],
                                    at[:],
                                    start=(k == 0),
                                    stop=(k == nsc - 1),
                                )
                            den_sb = spool.tile([1, 512], F32, name="den", tag="den")
                            nc.vector.tensor_copy(den_sb[:], ps_d[:])
                            yT_sb = spool.tile([P, 512], BF, name="yT", tag="yT")
                            nc.scalar.copy(yT_sb[:], ps_y[:])
                            for u in range(4):
                                t0 = tj * 512 + u * P
                                ps_dt = psdt.tile([P, 1], F32, name="ps_dt", tag="psdt")
                                nc.tensor.transpose(
                                    ps_dt[:], den_sb[:, u * P:(u + 1) * P],
                                    id_f32[:1, :1],
                                )
                                rec = spool.tile([P, 1], F32, name="rec", tag="rec")
                                nc.vector.reciprocal(rec[:], ps_dt[:])
                                ps_yt = psyt.tile([P, P], BF, name="ps_yt", tag="psyt")
                                nc.tensor.transpose(
                                    ps_yt[:], yT_sb[:, u * P:(u + 1) * P], id_bf[:]
                                )
                                o_sb = opool.tile([P, HS], F32, name="o_sb", tag="o")
                                nc.scalar.activation(
                                    o_sb[:], ps_yt[:], Copy, scale=rec[:]
                                )
                                nc.sync.dma_start(
                                    out[t0:t0 + P, h * HS:(h + 1) * HS], o_sb[:]
                                )
    nc.finalize()
    return nc


_ROPE_PERM = np.concatenate([np.arange(0, DHR, 2), np.arange(1, DHR, 2)])


def _bf(a):
    return np.ascontiguousarray(a).astype(ml_dtypes.bfloat16)


def _prep_inputs(x, freqs_cos, freqs_sin, W_dq, W_uq, W_dkv, W_uk, W_uv, W_qr,
                 W_kr, W_o):
    """Build the 8 per-core input maps (host-side layout prep, all bf16)."""
    x2 = np.asarray(x, np.float32).reshape(T, C)
    xT = x2.T                                        # [C, T]
    wdqT = _bf(np.asarray(W_dq).T.reshape(C, LQ // 4, 512).transpose(1, 0, 2))
    wdkvT = _bf(np.asarray(W_dkv).T.reshape(C, 1, 512).transpose(1, 0, 2))
    wkrT = _bf(np.asarray(W_kr)[_ROPE_PERM, :].T)    # [C, DHR], rope-permuted
    cosT = np.asarray(freqs_cos, np.float32).T       # [32, T]
    sinT = np.asarray(freqs_sin, np.float32).T
    cos2T = _bf(np.concatenate([cosT, cosT], axis=0))    # [64, T]
    sin2T = _bf(np.concatenate([-sinT, sinT], axis=0))
    wuq_full = np.asarray(W_uq).reshape(NLQ, NH * HS)
    wuv = _bf(np.asarray(W_uv).reshape(CCH, P, NLKV))
    W_qr_a = np.asarray(W_qr)
    W_uk_a = np.asarray(W_uk)
    W_o_a = np.asarray(W_o)

    in_maps = []
    for i in range(NCORES):
        h0 = i * HPC
        cols = slice(h0 * HS, (h0 + HPC) * HS)       # 256 output cols
        wqr_rows = np.concatenate(
            [W_qr_a[(h0 + h) * DHR + _ROPE_PERM, :] for h in range(HPC)], axis=0
        )                                            # [HPC*64=128, NLQ]
        in_maps.append({
            "xT_sh": _bf(xT[:, i * TS:(i + 1) * TS]),
            "wdqT": wdqT,
            "wdkvT": wdkvT,
            "wkrT": wkrT,
            "cos2T": cos2T,
            "sin2T": sin2T,
            "wuq": _bf(np.ascontiguousarray(wuq_full[:, cols])
                       .reshape(LQ, P, HPC * HS)),
            "wqrT": _bf(np.ascontiguousarray(wqr_rows.T)
                        .reshape(LQ, P, HPC * DHR)),
            "wukT": _bf(np.ascontiguousarray(
                        W_uk_a[h0 * HS:(h0 + HPC) * HS, :].T)
                        .reshape(LKV, P, HPC * HS)),
            "wuv": wuv,
            "woT": _bf(np.ascontiguousarray(W_o_a[cols, :].T)
                       .reshape(CCH, P, HPC * HS)),
        })
    return in_maps


_NC_CACHE = None


def kernel(**inputs):
    global _NC_CACHE
    in_maps = _prep_inputs(**inputs)
    if _NC_CACHE is None:
        _NC_CACHE = build_nc()
    res = run_bass_kernel_spmd(_NC_CACHE, in_maps, core_ids=list(range(NCORES)))
    outs = [np.asarray(res.results[i]["out"], np.float32) for i in range(NCORES)]
    y = np.concatenate(outs, axis=1).reshape(B, T, C)
    return y


# revision 14
# speedup vs baseline: 1.0321x; 1.0321x over previous
"""MLA-style attention (nn_Attention_7868380086611) on 8 TRN2 NeuronCores.

Strategy
--------
The reference "absorbs" the up-projections (k_eff = Wuq_h @ Wuk_h per head,
v_eff = (W_uv.T @ W_o.T) per-head slices), which is ~4x more FLOPs than the
factored form.  By matmul associativity we instead compute standard per-head
q/k (head dim 128) plus the decoupled-RoPE part, and an effective per-head
v~_h = c_kv @ (W_uv.T @ W_o.T)[:, cols_h], so the [T,T] attention matrix only
ever multiplies 128-wide tensors.

Sharding: head-parallel attention (2 of 16 heads per core) on top of
T-sharded down-projections.  Each core computes c_q/c_kv/k_r for its T/8
token slice (transposed layout, contraction dims on partitions), then one
AllGather (~1 MB/rank, bf16) replicates the tiny latents, and each core runs
the full causal attention for its 2 heads, writing its own 256 output
columns.  All inputs are pre-cast/pre-tiled to bf16 on the host; PSUM
accumulation is fp32.

The same SPMD graph runs on all 8 cores; all rank-dependence is carried by
the per-core input slices.
"""

import math
import sys

import numpy as np

sys.path.insert(0, "/opt/trn_rl_repo")

import ml_dtypes  # noqa: E402

from concourse import bacc, bass, masks, mybir  # noqa: E402
from concourse.bass_utils import run_bass_kernel_spmd  # noqa: E402
from concourse.tile import TileContext  # noqa: E402

B, T, C = 1, 2048, 2048
NH, HS = 16, 128
NLQ, NLKV, DHR = 1536, 512, 64
NCORES = 8
HPC = NH // NCORES          # heads per core = 2
TS = T // NCORES            # 256-token shard for down-projections
P = 128
LQ = NLQ // P               # 12 l-chunks
LKV = NLKV // P             # 4
CCH = C // P                # 16 c-chunks
TJ = T // 512               # 4 t-chunks of 512
SC = T // P                 # 16 s-chunks
SCALE = 1.0 / math.sqrt(HS + DHR)
NEG = -1.0e10

BF = mybir.dt.bfloat16
F32 = mybir.dt.float32
Exp = mybir.ActivationFunctionType.Exp
Copy = mybir.ActivationFunctionType.Copy

GROUP = NLQ + NLKV + DHR    # 2112 rows in the all-gather buffer


def build_nc():
    nc = bacc.Bacc(None, target_bir_lowering=False, num_devices=NCORES)

    xT_sh = nc.declare_dram_parameter("xT_sh", [C, TS], BF, isOutput=False)
    wdqT = nc.declare_dram_parameter("wdqT", [LQ // 4, C, 512], BF, isOutput=False)
    wdkvT = nc.declare_dram_parameter("wdkvT", [1, C, 512], BF, isOutput=False)
    wkrT = nc.declare_dram_parameter("wkrT", [C, DHR], BF, isOutput=False)
    cos2T = nc.declare_dram_parameter("cos2T", [DHR, T], BF, isOutput=False)
    sin2T = nc.declare_dram_parameter("sin2T", [DHR, T], BF, isOutput=False)
    wuq = nc.declare_dram_parameter("wuq", [LQ, P, HPC * HS], BF, isOutput=False)
    wqrT = nc.declare_dram_parameter("wqrT", [LQ, P, HPC * DHR], BF, isOutput=False)
    wukT = nc.declare_dram_parameter("wukT", [LKV, P, HPC * HS], BF, isOutput=False)
    wuv = nc.declare_dram_parameter("wuv", [CCH, P, NLKV], BF, isOutput=False)
    woT = nc.declare_dram_parameter("woT", [CCH, P, HPC * HS], BF, isOutput=False)
    out = nc.declare_dram_parameter("out", [T, HPC * HS], F32, isOutput=True)

    GKV = NLKV + DHR
    cc_in_kv = nc.dram_tensor("cc_in_kv", [GKV, TS], BF)
    cc_out_kv = nc.dram_tensor("cc_out_kv", [NCORES, GKV, TS], BF,
                               addr_space="Shared")
    cc_in_q = nc.dram_tensor("cc_in_q", [NLQ, TS], BF)
    cc_out_q = nc.dram_tensor("cc_out_q", [NCORES, NLQ, TS], BF,
                              addr_space="Shared")

    with TileContext(nc) as tc:
        with (
            tc.tile_pool(name="persist", bufs=1) as persist,
            tc.tile_pool(name="lat", bufs=1) as lat,
            tc.tile_pool(name="proj", bufs=1) as proj,
            tc.tile_pool(name="wts", bufs=1) as wts,
        ):
            # ---- constants ----
            id_bf = persist.tile([P, P], BF)
            masks.make_identity(nc, id_bf[:])
            id_f32 = persist.tile([P, P], F32)
            masks.make_identity(nc, id_f32[:])
            ones_bf = persist.tile([P, 1], BF)
            nc.vector.memset(ones_bf[:], 1.0)
            # 4 additive causal masks [128, 512]: keep (0) iff t - s - 128*m >= 0
            cmask = persist.tile([P, 4 * 512], F32)
            nc.gpsimd.memset(cmask[:], 0.0)
            for m in range(4):
                nc.gpsimd.affine_select(
                    out=cmask[:, m * 512:(m + 1) * 512],
                    in_=cmask[:, m * 512:(m + 1) * 512],
                    compare_op=mybir.AluOpType.is_ge,
                    fill=NEG,
                    base=-m * P,
                    channel_multiplier=-1,
                    pattern=[[1, 512]],
                )
            cos_sb = persist.tile([DHR, T], BF)
            nc.sync.dma_start(cos_sb[:], cos2T[:, :])
            sin_sb = persist.tile([DHR, T], BF)
            nc.sync.dma_start(sin_sb[:], sin2T[:, :])

            # ---- prefetch the post-gather projection weights on the scalar
            # queue so they never sit behind collective-gated DMAs ----
            wuq_all = wts.tile([P, LQ * HPC * HS], BF)
            for l in range(LQ):
                nc.scalar.dma_start(
                    wuq_all[:, l * HPC * HS:(l + 1) * HPC * HS], wuq[l]
                )
            wqr_all = wts.tile([P, LQ * HPC * DHR], BF)
            for l in range(LQ):
                nc.scalar.dma_start(
                    wqr_all[:, l * HPC * DHR:(l + 1) * HPC * DHR], wqrT[l]
                )
            wuk_all = wts.tile([P, LKV * HPC * HS], BF)
            for l in range(LKV):
                nc.scalar.dma_start(
                    wuk_all[:, l * HPC * HS:(l + 1) * HPC * HS], wukT[l]
                )

            # ---- phase 1: c_kv^T + k_r^T shard -> AG1; c_q^T shard -> AG2 ----
            with (
                tc.tile_pool(name="p1w", bufs=2) as p1w,
                tc.tile_pool(name="p1ps", bufs=2, space="PSUM") as p1ps,
                tc.tile_pool(name="p1sh", bufs=3) as p1sh,
            ):
                xt = []
                for g in range(4):
                    t = lat.tile([P, 4 * TS], BF, name=f"xt{g}", tag=f"xt{g}")
                    nc.sync.dma_start(
                        t[:].rearrange("p (n u) -> p n u", n=4),
                        xT_sh.ap()
                        .rearrange("(n p) u -> n p u", p=P)[4 * g:4 * (g + 1)]
                        .rearrange("n p u -> p n u"),
                    )
                    xt.append(t)

                def xtile(c):
                    return xt[c // 4][:, (c % 4) * TS:(c % 4 + 1) * TS]

                def down_proj(wparam, group, nsub, bounce, row0):
                    w = p1w.tile([P, CCH * nsub * P], BF, name="p1w_t", tag="p1w_t")
                    nc.sync.dma_start(
                        w[:].rearrange("p (n m) -> p n m", n=CCH),
                        wparam[group].rearrange("(n p) m -> p n m", p=P),
                    )
                    for ls in range(nsub):
                        ps = p1ps.tile([P, TS], F32, name="p1ps_t", tag="p1ps_t")
                        for c in range(CCH):
                            nc.tensor.matmul(
                                ps[:],
                                w[:, c * nsub * P + ls * P:
                                  c * nsub * P + (ls + 1) * P],
                                xtile(c),
                                start=(c == 0),
                                stop=(c == CCH - 1),
                            )
                        sh = p1sh.tile([P, TS], BF, name="p1sh_t", tag="p1sh_t")
                        nc.scalar.copy(sh[:], ps[:])
                        nc.scalar.dma_start(
                            bounce[row0 + ls * P: row0 + (ls + 1) * P, :], sh[:]
                        )

                # c_kv (4 l-chunks) then k_r, then AG1
                down_proj(wdkvT, 0, 4, cc_in_kv, 0)
                wkr_sb = p1w.tile([P, CCH * DHR], BF, name="wkr_sb")
                nc.sync.dma_start(
                    wkr_sb[:].rearrange("p (n m) -> p n m", n=CCH),
                    wkrT.ap().rearrange("(n p) m -> p n m", p=P),
                )
                ps_kr = p1ps.tile([DHR, TS], F32, name="ps_kr", tag="p1ps_t")
                for c in range(CCH):
                    nc.tensor.matmul(
                        ps_kr[:],
                        wkr_sb[:, c * DHR:(c + 1) * DHR],
                        xtile(c),
                        start=(c == 0),
                        stop=(c == CCH - 1),
                    )
                sh_kr = p1sh.tile([DHR, TS], BF, name="sh_kr")
                nc.scalar.copy(sh_kr[:], ps_kr[:])
                nc.scalar.dma_start(cc_in_kv[NLKV:GKV, :], sh_kr[:])

                nc.gpsimd.collective_compute(
                    "AllGather",
                    mybir.AluOpType.bypass,
                    replica_groups=[list(range(NCORES))],
                    ins=[cc_in_kv.ap().opt()],
                    outs=[cc_out_kv.ap().opt()],
                )

                # c_q (12 l-chunks in 3 groups of 4), then AG2
                for g in range(LQ // 4):
                    down_proj(wdqT, g, 4, cc_in_q, g * 4 * P)

            nc.gpsimd.collective_compute(
                "AllGather",
                mybir.AluOpType.bypass,
                replica_groups=[list(range(NCORES))],
                ins=[cc_in_q.ap().opt()],
                outs=[cc_out_q.ap().opt()],
            )

            # ---- B = (W_uv.T @ W_o.T)[:, 2-head cols]  (independent of AGs) ----
            b_all = proj.tile([P, LKV * HPC * HS], BF)  # [128, 4*256]
            with (
                tc.tile_pool(name="pbw", bufs=3) as pbw,
                tc.tile_pool(name="pbps", bufs=1, space="PSUM") as pbps,
            ):
                ps_b = [
                    pbps.tile([P, HPC * HS], F32, name=f"ps_b{m}") for m in range(LKV)
                ]
                for c in range(CCH):
                    wuv_t = pbw.tile([P, NLKV], BF, name="wuv_t", tag="wuv_t")
                    nc.sync.dma_start(wuv_t[:], wuv[c])
                    wo_t = pbw.tile([P, HPC * HS], BF, name="wo_t", tag="wo_t")
                    nc.sync.dma_start(wo_t[:], woT[c])
                    for m in range(LKV):
                        nc.tensor.matmul(
                            ps_b[m][:],
                            wuv_t[:, m * P:(m + 1) * P],
                            wo_t[:],
                            start=(c == 0),
                            stop=(c == CCH - 1),
                        )
                for m in range(LKV):
                    nc.vector.tensor_copy(
                        b_all[:, m * HPC * HS:(m + 1) * HPC * HS], ps_b[m][:]
                    )

            # ---- gathered kv latents (arrive first) ----
            ckv_t = []
            for l in range(LKV):
                t = lat.tile([P, T], BF, name=f"ckv{l}", tag=f"ckv{l}")
                nc.sync.dma_start(
                    t[:].rearrange("p (g u) -> p g u", g=NCORES),
                    cc_out_kv[:, l * P:(l + 1) * P, :].rearrange("g p u -> p g u"),
                )
                ckv_t.append(t)
            kr_raw = lat.tile([DHR, T], BF)
            nc.sync.dma_start(
                kr_raw[:].rearrange("p (g u) -> p g u", g=NCORES),
                cc_out_kv[:, NLKV:GKV, :].rearrange("g p u -> p g u"),
            )

            # ---- rope on k_r ----
            kr_rope = proj.tile([DHR, T], BF)
            with tc.tile_pool(name="rtmp", bufs=2) as rtmp:

                def rope(dst, src):
                    # dst = src * [cos;cos] + swap_halves(src) * [-sin;sin]
                    sw = rtmp.tile([DHR, T], BF, name="rsw", tag="rsw")
                    nc.sync.dma_start(sw[0:32, :], src[32:64, :])
                    nc.sync.dma_start(sw[32:64, :], src[0:32, :])
                    ta = rtmp.tile([DHR, T], BF, name="rta", tag="rta")
                    tb = rtmp.tile([DHR, T], BF, name="rtb", tag="rtb")
                    nc.vector.tensor_mul(ta[:], src, cos_sb[:])
                    nc.vector.tensor_mul(tb[:], sw[:], sin_sb[:])
                    nc.vector.tensor_add(dst, ta[:], tb[:])

                rope(kr_rope[:, :], kr_raw[:, :])

                # ---- projections, in data-arrival order: k, v~, then q-side ----
                qT = proj.tile([P, HPC * T], BF)
                kT = proj.tile([P, HPC * T], BF)
                qr_rope = proj.tile([DHR, HPC * T], BF)
                v_sb = proj.tile([P, SC * HPC * HS], BF)

                with tc.tile_pool(name="p5ps", bufs=4, space="PSUM") as p5ps:
                    # k^T per head
                    for h in range(HPC):
                        for sj in range(TJ):
                            ps = p5ps.tile([P, 512], F32, name="ps_k", tag="p5")
                            for l in range(LKV):
                                nc.tensor.matmul(
                                    ps[:],
                                    wuk_all[:, l * HPC * HS + h * HS:
                                            l * HPC * HS + (h + 1) * HS],
                                    ckv_t[l][:, sj * 512:(sj + 1) * 512],
                                    start=(l == 0),
                                    stop=(l == LKV - 1),
                                )
                            nc.vector.tensor_copy(
                                kT[:, h * T + sj * 512: h * T + (sj + 1) * 512],
                                ps[:],
                            )
                    # v~ per s-chunk
                    for sc in range(SC):
                        ps = p5ps.tile([P, HPC * HS], F32, name="ps_v", tag="p5")
                        for l in range(LKV):
                            nc.tensor.matmul(
                                ps[:],
                                ckv_t[l][:, sc * P:(sc + 1) * P],
                                b_all[:, l * HPC * HS:(l + 1) * HPC * HS],
                                start=(l == 0),
                                stop=(l == LKV - 1),
                            )
                        nc.vector.tensor_copy(
                            v_sb[:, sc * HPC * HS:(sc + 1) * HPC * HS], ps[:]
                        )

                    # gathered q latent (arrives second)
                    cq_t = []
                    for l in range(LQ):
                        t = lat.tile([P, T], BF, name=f"cq{l}", tag=f"cq{l}")
                        nc.sync.dma_start(
                            t[:].rearrange("p (g u) -> p g u", g=NCORES),
                            cc_out_q[:, l * P:(l + 1) * P, :].rearrange(
                                "g p u -> p g u"
                            ),
                        )
                        cq_t.append(t)

                    # q_r^T per head (head on free axis), then rope
                    qr_raw = proj.tile([DHR, HPC * T], BF)
                    for h in range(HPC):
                        for tj in range(TJ):
                            ps = p5ps.tile([DHR, 512], F32, name="ps_qr", tag="p5")
                            for l in range(LQ):
                                nc.tensor.matmul(
                                    ps[:],
                                    wqr_all[:, l * HPC * DHR + h * DHR:
                                            l * HPC * DHR + (h + 1) * DHR],
                                    cq_t[l][:, tj * 512:(tj + 1) * 512],
                                    start=(l == 0),
                                    stop=(l == LQ - 1),
                                )
                            nc.vector.tensor_copy(
                                qr_raw[:, h * T + tj * 512: h * T + (tj + 1) * 512],
                                ps[:],
                            )
                    for h in range(HPC):
                        rope(qr_rope[:, h * T:(h + 1) * T],
                             qr_raw[:, h * T:(h + 1) * T])

                    # q^T per head
                    for h in range(HPC):
                        for tj in range(TJ):
                            ps = p5ps.tile([P, 512], F32, name="ps_q", tag="p5")
                            for l in range(LQ):
                                nc.tensor.matmul(
                                    ps[:],
                                    wuq_all[:, l * HPC * HS + h * HS:
                                            l * HPC * HS + (h + 1) * HS],
                                    cq_t[l][:, tj * 512:(tj + 1) * 512],
                                    start=(l == 0),
                                    stop=(l == LQ - 1),
                                )
                            nc.vector.tensor_copy(
                                qT[:, h * T + tj * 512: h * T + (tj + 1) * 512],
                                ps[:],
                            )

                # ---- attention (causal, per head, transposed-scores flow) ----
                with (
                    tc.tile_pool(name="pss", bufs=3, space="PSUM") as pss,
                    tc.tile_pool(name="psy", bufs=2, space="PSUM") as psy,
                    tc.tile_pool(name="psd", bufs=1, space="PSUM") as psd,
                    tc.tile_pool(name="psdt", bufs=1, space="PSUM") as psdt,
                    tc.tile_pool(name="psyt", bufs=1, space="PSUM") as psyt,
                    tc.tile_pool(name="atp", bufs=6) as atp,
                    tc.tile_pool(name="spool", bufs=3) as spool,
                    tc.tile_pool(name="opool", bufs=3) as opool,
                ):
                    for h in range(HPC):
                        for tj in range(TJ):
                            nsc = 4 * (tj + 1)
                            ps_y = psy.tile([P, 512], F32, name="ps_y", tag="psy")
                            ps_d = psd.tile([1, 512], F32, name="ps_d", tag="psd")
                            for k in range(nsc):
                                ps_s = pss.tile([P, 512], F32, name="ps_s", tag="pss")
                                nc.tensor.matmul(
                                    ps_s[:],
                                    kT[:, h * T + k * P: h * T + (k + 1) * P],
                                    qT[:, h * T + tj * 512: h * T + (tj + 1) * 512],
                                    start=True,
                                    stop=False,
                                )
                                nc.tensor.matmul(
                                    ps_s[:],
                                    kr_rope[:, k * P:(k + 1) * P],
                                    qr_rope[:, h * T + tj * 512:
                                            h * T + (tj + 1) * 512],
                                    start=False,
                                    stop=True,
                                )
                                m = k - 4 * tj
                                if m >= 0:
                                    nc.vector.tensor_add(
                                        ps_s[:], ps_s[:],
                                        cmask[:, m * 512:(m + 1) * 512],
                                    )
                                at = atp.tile([P, 512], BF, name="at", tag="at")
                                nc.scalar.activation(at[:], ps_s[:], Exp, scale=SCALE)
                                nc.tensor.matmul(
                                    ps_y[:],
                                    v_sb[:, k * HPC * HS + h * HS:
                                         k * HPC * HS + (h + 1) * HS],
                                    at[:],
                                    start=(k == 0),
                                    stop=(k == nsc - 1),
                                )
                                nc.tensor.matmul(
                                    ps_d[:],
                                    ones_bf[:],
                                    at[:],
                                    start=(k == 0),
                                    stop=(k == nsc - 1),
                                )
                            den_sb = spool.tile([1, 512], F32, name="den", tag="den")
                            nc.vector.tensor_copy(den_sb[:], ps_d[:])
                            yT_sb = spool.tile([P, 512], BF, name="yT", tag="yT")
                            nc.scalar.copy(yT_sb[:], ps_y[:])
                            for u in range(4):
                                t0 = tj * 512 + u * P
                                ps_dt = psdt.tile([P, 1], F32, name="ps_dt",
                                                  tag="psdt")
                                nc.tensor.transpose(
                                    ps_dt[:], den_sb[:, u * P:(u + 1) * P],
                                    id_f32[:1, :1],
                                )
                                rec = spool.tile([P, 1], F32, name="rec", tag="rec")
                                nc.vector.reciprocal(rec[:], ps_dt[:])
                                ps_yt = psyt.tile([P, P], BF, name="ps_yt",
                                                  tag="psyt")
                                nc.tensor.transpose(
                                    ps_yt[:], yT_sb[:, u * P:(u + 1) * P], id_bf[:]
                                )
                                o_sb = opool.tile([P, HS], F32, name="o_sb", tag="o")
                                nc.scalar.activation(
                                    o_sb[:], ps_yt[:], Copy, scale=rec[:]
                                )
                                nc.sync.dma_start(
                                    out[t0:t0 + P, h * HS:(h + 1) * HS], o_sb[:]
                                )
    nc.finalize()
    return nc


_ROPE_PERM = np.concatenate([np.arange(0, DHR, 2), np.arange(1, DHR, 2)])


def _bf(a):
    return np.ascontiguousarray(a).astype(ml_dtypes.bfloat16)


def _prep_inputs(x, freqs_cos, freqs_sin, W_dq, W_uq, W_dkv, W_uk, W_uv, W_qr,
                 W_kr, W_o):
    """Build the 8 per-core input maps (host-side layout prep, all bf16)."""
    x2 = np.asarray(x, np.float32).reshape(T, C)
    xT = x2.T                                        # [C, T]
    wdqT = _bf(np.asarray(W_dq).T.reshape(C, LQ // 4, 512).transpose(1, 0, 2))
    wdkvT = _bf(np.asarray(W_dkv).T.reshape(C, 1, 512).transpose(1, 0, 2))
    wkrT = _bf(np.asarray(W_kr)[_ROPE_PERM, :].T)    # [C, DHR], rope-permuted
    cosT = np.asarray(freqs_cos, np.float32).T       # [32, T]
    sinT = np.asarray(freqs_sin, np.float32).T
    cos2T = _bf(np.concatenate([cosT, cosT], axis=0))    # [64, T]
    sin2T = _bf(np.concatenate([-sinT, sinT], axis=0))
    wuq_full = np.asarray(W_uq).reshape(NLQ, NH * HS)
    wuv = _bf(np.asarray(W_uv).reshape(CCH, P, NLKV))
    W_qr_a = np.asarray(W_qr)
    W_uk_a = np.asarray(W_uk)
    W_o_a = np.asarray(W_o)

    in_maps = []
    for i in range(NCORES):
        h0 = i * HPC
        cols = slice(h0 * HS, (h0 + HPC) * HS)       # 256 output cols
        wqr_rows = np.concatenate(
            [W_qr_a[(h0 + h) * DHR + _ROPE_PERM, :] for h in range(HPC)], axis=0
        )                                            # [HPC*64=128, NLQ]
        in_maps.append({
            "xT_sh": _bf(xT[:, i * TS:(i + 1) * TS]),
            "wdqT": wdqT,
            "wdkvT": wdkvT,
            "wkrT": wkrT,
            "cos2T": cos2T,
            "sin2T": sin2T,
            "wuq": _bf(np.ascontiguousarray(wuq_full[:, cols])
                       .reshape(LQ, P, HPC * HS)),
            "wqrT": _bf(np.ascontiguousarray(wqr_rows.T)
                        .reshape(LQ, P, HPC * DHR)),
            "wukT": _bf(np.ascontiguousarray(
                        W_uk_a[h0 * HS:(h0 + HPC) * HS, :].T)
                        .reshape(LKV, P, HPC * HS)),
            "wuv": wuv,
            "woT": _bf(np.ascontiguousarray(W_o_a[cols, :].T)
                       .reshape(CCH, P, HPC * HS)),
        })
    return in_maps


_NC_CACHE = None


def kernel(**inputs):
    global _NC_CACHE
    in_maps = _prep_inputs(**inputs)
    if _NC_CACHE is None:
        _NC_CACHE = build_nc()
    res = run_bass_kernel_spmd(_NC_CACHE, in_maps, core_ids=list(range(NCORES)))
    outs = [np.asarray(res.results[i]["out"], np.float32) for i in range(NCORES)]
    y = np.concatenate(outs, axis=1).reshape(B, T, C)
    return y


# revision 15
# speedup vs baseline: 1.1057x; 1.0713x over previous
"""MLA-style attention (nn_Attention_7868380086611) on 8 TRN2 NeuronCores.

Strategy
--------
The reference "absorbs" the up-projections (k_eff = Wuq_h @ Wuk_h per head,
v_eff = (W_uv.T @ W_o.T) per-head slices), which is ~4x more FLOPs than the
factored form.  By matmul associativity we instead compute standard per-head
q/k (head dim 128) plus the decoupled-RoPE part, and an effective per-head
v~_h = c_kv @ (W_uv.T @ W_o.T)[:, cols_h], so the [T,T] attention matrix only
ever multiplies 128-wide tensors.

Sharding: head-parallel attention (2 of 16 heads per core) on top of
T-sharded down-projections.  Each core computes c_q/c_kv/k_r for its T/8
token slice (transposed layout, contraction dims on partitions), then one
AllGather (~1 MB/rank, bf16) replicates the tiny latents, and each core runs
the full causal attention for its 2 heads, writing its own 256 output
columns.  All inputs are pre-cast/pre-tiled to bf16 on the host; PSUM
accumulation is fp32.

The same SPMD graph runs on all 8 cores; all rank-dependence is carried by
the per-core input slices.
"""

import math
import sys

import numpy as np

sys.path.insert(0, "/opt/trn_rl_repo")

import ml_dtypes  # noqa: E402

from concourse import bacc, bass, masks, mybir  # noqa: E402
from concourse.bass_utils import run_bass_kernel_spmd  # noqa: E402
from concourse.tile import TileContext  # noqa: E402

B, T, C = 1, 2048, 2048
NH, HS = 16, 128
NLQ, NLKV, DHR = 1536, 512, 64
NCORES = 8
HPC = NH // NCORES          # heads per core = 2
TS = T // NCORES            # 256-token shard for down-projections
P = 128
LQ = NLQ // P               # 12 l-chunks
LKV = NLKV // P             # 4
CCH = C // P                # 16 c-chunks
TJ = T // 512               # 4 t-chunks of 512
SC = T // P                 # 16 s-chunks
SCALE = 1.0 / math.sqrt(HS + DHR)
NEG = -1.0e10

BF = mybir.dt.bfloat16
F32 = mybir.dt.float32
Exp = mybir.ActivationFunctionType.Exp
Copy = mybir.ActivationFunctionType.Copy

GROUP = NLQ + NLKV + DHR    # 2112 rows in the all-gather buffer


def build_nc():
    nc = bacc.Bacc(None, target_bir_lowering=False, num_devices=NCORES)

    xT_sh = nc.declare_dram_parameter("xT_sh", [C, TS], BF, isOutput=False)
    wdqT = nc.declare_dram_parameter("wdqT", [LQ // 4, C, 512], BF, isOutput=False)
    wdkvT = nc.declare_dram_parameter("wdkvT", [1, C, 512], BF, isOutput=False)
    wkrT = nc.declare_dram_parameter("wkrT", [C, DHR], BF, isOutput=False)
    cos2T = nc.declare_dram_parameter("cos2T", [DHR, T], BF, isOutput=False)
    sin2T = nc.declare_dram_parameter("sin2T", [DHR, T], BF, isOutput=False)
    wuq = nc.declare_dram_parameter("wuq", [LQ, P, HPC * HS], BF, isOutput=False)
    wqrT = nc.declare_dram_parameter("wqrT", [LQ, P, HPC * DHR], BF, isOutput=False)
    wukT = nc.declare_dram_parameter("wukT", [LKV, P, HPC * HS], BF, isOutput=False)
    wuv = nc.declare_dram_parameter("wuv", [CCH, P, NLKV], BF, isOutput=False)
    woT = nc.declare_dram_parameter("woT", [CCH, P, HPC * HS], BF, isOutput=False)
    out = nc.declare_dram_parameter("out", [T, HPC * HS], F32, isOutput=True)

    GKV = NLKV + DHR
    cc_in_kv = nc.dram_tensor("cc_in_kv", [GKV, TS], BF)
    cc_out_kv = nc.dram_tensor("cc_out_kv", [NCORES, GKV, TS], BF,
                               addr_space="Shared")
    cc_in_q = nc.dram_tensor("cc_in_q", [NLQ, TS], BF)
    cc_out_q = nc.dram_tensor("cc_out_q", [NCORES, NLQ, TS], BF,
                              addr_space="Shared")

    with TileContext(nc) as tc:
        with (
            tc.tile_pool(name="persist", bufs=1) as persist,
            tc.tile_pool(name="lat", bufs=1) as lat,
            tc.tile_pool(name="proj", bufs=1) as proj,
            tc.tile_pool(name="wts", bufs=1) as wts,
        ):
            # ---- constants ----
            id_bf = persist.tile([P, P], BF)
            masks.make_identity(nc, id_bf[:])
            id_f32 = persist.tile([P, P], F32)
            masks.make_identity(nc, id_f32[:])
            ones_bf = persist.tile([P, 1], BF)
            nc.vector.memset(ones_bf[:], 1.0)
            # 4 additive causal masks [128, 512]: keep (0) iff t - s - 128*m >= 0
            cmask = persist.tile([P, 4 * 512], F32)
            nc.gpsimd.memset(cmask[:], 0.0)
            for m in range(4):
                nc.gpsimd.affine_select(
                    out=cmask[:, m * 512:(m + 1) * 512],
                    in_=cmask[:, m * 512:(m + 1) * 512],
                    compare_op=mybir.AluOpType.is_ge,
                    fill=NEG,
                    base=-m * P,
                    channel_multiplier=-1,
                    pattern=[[1, 512]],
                )
            cos_sb = persist.tile([DHR, T], BF)
            nc.sync.dma_start(cos_sb[:], cos2T[:, :])
            sin_sb = persist.tile([DHR, T], BF)
            nc.sync.dma_start(sin_sb[:], sin2T[:, :])

            # ---- prefetch the post-gather projection weights on the scalar
            # queue so they never sit behind collective-gated DMAs ----
            wuq_all = wts.tile([P, LQ * HPC * HS], BF)
            for l in range(LQ):
                nc.scalar.dma_start(
                    wuq_all[:, l * HPC * HS:(l + 1) * HPC * HS], wuq[l]
                )
            wqr_all = wts.tile([P, LQ * HPC * DHR], BF)
            for l in range(LQ):
                nc.scalar.dma_start(
                    wqr_all[:, l * HPC * DHR:(l + 1) * HPC * DHR], wqrT[l]
                )
            wuk_all = wts.tile([P, LKV * HPC * HS], BF)
            for l in range(LKV):
                nc.scalar.dma_start(
                    wuk_all[:, l * HPC * HS:(l + 1) * HPC * HS], wukT[l]
                )

            # ---- phase 1: c_q^T shard -> AG-q; then c_kv^T/k_r^T -> AG-kv.
            # AG-q goes first: its dependent work (q/qr proj, 74k PE rows) is
            # 2.3x the kv-side's, so the kv collective hides under it.
            with (
                tc.tile_pool(name="p1w", bufs=2) as p1w,
                tc.tile_pool(name="p1ps", bufs=2, space="PSUM") as p1ps,
                tc.tile_pool(name="p1sh", bufs=3) as p1sh,
            ):
                xt = []
                for g in range(4):
                    t = lat.tile([P, 4 * TS], BF, name=f"xt{g}", tag=f"xt{g}")
                    nc.sync.dma_start(
                        t[:].rearrange("p (n u) -> p n u", n=4),
                        xT_sh.ap()
                        .rearrange("(n p) u -> n p u", p=P)[4 * g:4 * (g + 1)]
                        .rearrange("n p u -> p n u"),
                    )
                    xt.append(t)

                def xtile(c):
                    return xt[c // 4][:, (c % 4) * TS:(c % 4 + 1) * TS]

                def down_proj(wparam, group, nsub, bounce, row0):
                    w = p1w.tile([P, CCH * nsub * P], BF, name="p1w_t", tag="p1w_t")
                    nc.sync.dma_start(
                        w[:].rearrange("p (n m) -> p n m", n=CCH),
                        wparam[group].rearrange("(n p) m -> p n m", p=P),
                    )
                    for ls in range(nsub):
                        ps = p1ps.tile([P, TS], F32, name="p1ps_t", tag="p1ps_t")
                        for c in range(CCH):
                            nc.tensor.matmul(
                                ps[:],
                                w[:, c * nsub * P + ls * P:
                                  c * nsub * P + (ls + 1) * P],
                                xtile(c),
                                start=(c == 0),
                                stop=(c == CCH - 1),
                            )
                        sh = p1sh.tile([P, TS], BF, name="p1sh_t", tag="p1sh_t")
                        nc.scalar.copy(sh[:], ps[:])
                        nc.scalar.dma_start(
                            bounce[row0 + ls * P: row0 + (ls + 1) * P, :], sh[:]
                        )

                # c_q (12 l-chunks in 3 groups of 4), then AG-q
                for g in range(LQ // 4):
                    down_proj(wdqT, g, 4, cc_in_q, g * 4 * P)
                nc.gpsimd.collective_compute(
                    "AllGather",
                    mybir.AluOpType.bypass,
                    replica_groups=[list(range(NCORES))],
                    ins=[cc_in_q.ap().opt()],
                    outs=[cc_out_q.ap().opt()],
                )

                # c_kv (4 l-chunks) then k_r, then AG-kv
                down_proj(wdkvT, 0, 4, cc_in_kv, 0)
                wkr_sb = p1w.tile([P, CCH * DHR], BF, name="wkr_sb")
                nc.sync.dma_start(
                    wkr_sb[:].rearrange("p (n m) -> p n m", n=CCH),
                    wkrT.ap().rearrange("(n p) m -> p n m", p=P),
                )
                ps_kr = p1ps.tile([DHR, TS], F32, name="ps_kr", tag="p1ps_t")
                for c in range(CCH):
                    nc.tensor.matmul(
                        ps_kr[:],
                        wkr_sb[:, c * DHR:(c + 1) * DHR],
                        xtile(c),
                        start=(c == 0),
                        stop=(c == CCH - 1),
                    )
                sh_kr = p1sh.tile([DHR, TS], BF, name="sh_kr")
                nc.scalar.copy(sh_kr[:], ps_kr[:])
                nc.scalar.dma_start(cc_in_kv[NLKV:GKV, :], sh_kr[:])

                nc.gpsimd.collective_compute(
                    "AllGather",
                    mybir.AluOpType.bypass,
                    replica_groups=[list(range(NCORES))],
                    ins=[cc_in_kv.ap().opt()],
                    outs=[cc_out_kv.ap().opt()],
                )

            # ---- B = (W_uv.T @ W_o.T)[:, 2-head cols]  (independent of AGs) ----
            b_all = proj.tile([P, LKV * HPC * HS], BF)  # [128, 4*256]
            with (
                tc.tile_pool(name="pbw", bufs=3) as pbw,
                tc.tile_pool(name="pbps", bufs=1, space="PSUM") as pbps,
            ):
                ps_b = [
                    pbps.tile([P, HPC * HS], F32, name=f"ps_b{m}") for m in range(LKV)
                ]
                for c in range(CCH):
                    wuv_t = pbw.tile([P, NLKV], BF, name="wuv_t", tag="wuv_t")
                    nc.sync.dma_start(wuv_t[:], wuv[c])
                    wo_t = pbw.tile([P, HPC * HS], BF, name="wo_t", tag="wo_t")
                    nc.sync.dma_start(wo_t[:], woT[c])
                    for m in range(LKV):
                        nc.tensor.matmul(
                            ps_b[m][:],
                            wuv_t[:, m * P:(m + 1) * P],
                            wo_t[:],
                            start=(c == 0),
                            stop=(c == CCH - 1),
                        )
                for m in range(LKV):
                    nc.vector.tensor_copy(
                        b_all[:, m * HPC * HS:(m + 1) * HPC * HS], ps_b[m][:]
                    )

            # ---- gathered q latent (arrives first) ----
            cq_t = []
            for l in range(LQ):
                t = lat.tile([P, T], BF, name=f"cq{l}", tag=f"cq{l}")
                nc.sync.dma_start(
                    t[:].rearrange("p (g u) -> p g u", g=NCORES),
                    cc_out_q[:, l * P:(l + 1) * P, :].rearrange("g p u -> p g u"),
                )
                cq_t.append(t)

            with tc.tile_pool(name="rtmp", bufs=2) as rtmp:

                def rope(dst, src):
                    # dst = src * [cos;cos] + swap_halves(src) * [-sin;sin]
                    sw = rtmp.tile([DHR, T], BF, name="rsw", tag="rsw")
                    nc.sync.dma_start(sw[0:32, :], src[32:64, :])
                    nc.sync.dma_start(sw[32:64, :], src[0:32, :])
                    ta = rtmp.tile([DHR, T], BF, name="rta", tag="rta")
                    tb = rtmp.tile([DHR, T], BF, name="rtb", tag="rtb")
                    nc.vector.tensor_mul(ta[:], src, cos_sb[:])
                    nc.vector.tensor_mul(tb[:], sw[:], sin_sb[:])
                    nc.vector.tensor_add(dst, ta[:], tb[:])

                qT = proj.tile([P, HPC * T], BF)
                kT = proj.tile([P, HPC * T], BF)
                qr_rope = proj.tile([DHR, HPC * T], BF)
                qr_raw = proj.tile([DHR, HPC * T], BF)
                qr2 = proj.tile([P, T], BF)          # merged 2-head qr, pre-split
                v_sb = proj.tile([P, SC * HPC * HS], BF)
                kr_rope = proj.tile([DHR, T], BF)

                with tc.tile_pool(name="p5ps", bufs=3, space="PSUM") as p5ps:
                    # q_r^T both heads in one matmul (M=128), split after
                    for tj in range(TJ):
                        ps = p5ps.tile([P, 512], F32, name="ps_qr", tag="p5")
                        for l in range(LQ):
                            nc.tensor.matmul(
                                ps[:],
                                wqr_all[:, l * HPC * DHR:(l + 1) * HPC * DHR],
                                cq_t[l][:, tj * 512:(tj + 1) * 512],
                                start=(l == 0),
                                stop=(l == LQ - 1),
                            )
                        nc.vector.tensor_copy(qr2[:, tj * 512:(tj + 1) * 512], ps[:])
                    nc.vector.tensor_copy(qr_raw[:, 0:T], qr2[0:DHR, :])
                    nc.sync.dma_start(qr_raw[:, T:HPC * T], qr2[DHR:P, :])
                    for h in range(HPC):
                        rope(qr_rope[:, h * T:(h + 1) * T],
                             qr_raw[:, h * T:(h + 1) * T])

                    # q^T per head
                    for h in range(HPC):
                        for tj in range(TJ):
                            ps = p5ps.tile([P, 512], F32, name="ps_q", tag="p5")
                            for l in range(LQ):
                                nc.tensor.matmul(
                                    ps[:],
                                    wuq_all[:, l * HPC * HS + h * HS:
                                            l * HPC * HS + (h + 1) * HS],
                                    cq_t[l][:, tj * 512:(tj + 1) * 512],
                                    start=(l == 0),
                                    stop=(l == LQ - 1),
                                )
                            nc.vector.tensor_copy(
                                qT[:, h * T + tj * 512: h * T + (tj + 1) * 512],
                                ps[:],
                            )

                    # gathered kv latents (arrive second)
                    ckv_t = []
                    for l in range(LKV):
                        t = lat.tile([P, T], BF, name=f"ckv{l}", tag=f"ckv{l}")
                        nc.sync.dma_start(
                            t[:].rearrange("p (g u) -> p g u", g=NCORES),
                            cc_out_kv[:, l * P:(l + 1) * P, :].rearrange(
                                "g p u -> p g u"
                            ),
                        )
                        ckv_t.append(t)
                    kr_raw = lat.tile([DHR, T], BF)
                    nc.sync.dma_start(
                        kr_raw[:].rearrange("p (g u) -> p g u", g=NCORES),
                        cc_out_kv[:, NLKV:GKV, :].rearrange("g p u -> p g u"),
                    )
                    rope(kr_rope[:, :], kr_raw[:, :])

                    # k^T per head
                    for h in range(HPC):
                        for sj in range(TJ):
                            ps = p5ps.tile([P, 512], F32, name="ps_k", tag="p5")
                            for l in range(LKV):
                                nc.tensor.matmul(
                                    ps[:],
                                    wuk_all[:, l * HPC * HS + h * HS:
                                            l * HPC * HS + (h + 1) * HS],
                                    ckv_t[l][:, sj * 512:(sj + 1) * 512],
                                    start=(l == 0),
                                    stop=(l == LKV - 1),
                                )
                            nc.vector.tensor_copy(
                                kT[:, h * T + sj * 512: h * T + (sj + 1) * 512],
                                ps[:],
                            )
                    # v~ per s-chunk
                    for sc in range(SC):
                        ps = p5ps.tile([P, HPC * HS], F32, name="ps_v", tag="p5")
                        for l in range(LKV):
                            nc.tensor.matmul(
                                ps[:],
                                ckv_t[l][:, sc * P:(sc + 1) * P],
                                b_all[:, l * HPC * HS:(l + 1) * HPC * HS],
                                start=(l == 0),
                                stop=(l == LKV - 1),
                            )
                        nc.vector.tensor_copy(
                            v_sb[:, sc * HPC * HS:(sc + 1) * HPC * HS], ps[:]
                        )

                # ---- attention (causal, per head, transposed-scores flow) ----
                with (
                    tc.tile_pool(name="pss", bufs=3, space="PSUM") as pss,
                    tc.tile_pool(name="psy", bufs=2, space="PSUM") as psy,
                    tc.tile_pool(name="psd", bufs=1, space="PSUM") as psd,
                    tc.tile_pool(name="psdt", bufs=1, space="PSUM") as psdt,
                    tc.tile_pool(name="psyt", bufs=1, space="PSUM") as psyt,
                    tc.tile_pool(name="atp", bufs=6) as atp,
                    tc.tile_pool(name="spool", bufs=3) as spool,
                    tc.tile_pool(name="opool", bufs=3) as opool,
                ):
                    for h in range(HPC):
                        for tj in range(TJ):
                            nsc = 4 * (tj + 1)
                            ps_y = psy.tile([P, 512], F32, name="ps_y", tag="psy")
                            ps_d = psd.tile([1, 512], F32, name="ps_d", tag="psd")
                            for k in range(nsc):
                                ps_s = pss.tile([P, 512], F32, name="ps_s", tag="pss")
                                nc.tensor.matmul(
                                    ps_s[:],
                                    kT[:, h * T + k * P: h * T + (k + 1) * P],
                                    qT[:, h * T + tj * 512: h * T + (tj + 1) * 512],
                                    start=True,
                                    stop=False,
                                )
                                nc.tensor.matmul(
                                    ps_s[:],
                                    kr_rope[:, k * P:(k + 1) * P],
                                    qr_rope[:, h * T + tj * 512:
                                            h * T + (tj + 1) * 512],
                                    start=False,
                                    stop=True,
                                )
                                m = k - 4 * tj
                                if m >= 0:
                                    nc.vector.tensor_add(
                                        ps_s[:], ps_s[:],
                                        cmask[:, m * 512:(m + 1) * 512],
                                    )
                                at = atp.tile([P, 512], BF, name="at", tag="at")
                                nc.scalar.activation(at[:], ps_s[:], Exp, scale=SCALE)
                                nc.tensor.matmul(
                                    ps_y[:],
                                    v_sb[:, k * HPC * HS + h * HS:
                                         k * HPC * HS + (h + 1) * HS],
                                    at[:],
                                    start=(k == 0),
                                    stop=(k == nsc - 1),
                                )
                                nc.tensor.matmul(
                                    ps_d[:],
                                    ones_bf[:],
                                    at[:],
                                    start=(k == 0),
                                    stop=(k == nsc - 1),
                                )
                            den_sb = spool.tile([1, 512], F32, name="den", tag="den")
                            nc.vector.tensor_copy(den_sb[:], ps_d[:])
                            yT_sb = spool.tile([P, 512], BF, name="yT", tag="yT")
                            nc.scalar.copy(yT_sb[:], ps_y[:])
                            for u in range(4):
                                t0 = tj * 512 + u * P
                                ps_dt = psdt.tile([P, 1], F32, name="ps_dt",
                                                  tag="psdt")
                                nc.tensor.transpose(
                                    ps_dt[:], den_sb[:, u * P:(u + 1) * P],
                                    id_f32[:1, :1],
                                )
                                rec = spool.tile([P, 1], F32, name="rec", tag="rec")
                                nc.vector.reciprocal(rec[:], ps_dt[:])
                                ps_yt = psyt.tile([P, P], BF, name="ps_yt",
                                                  tag="psyt")
                                nc.tensor.transpose(
                                    ps_yt[:], yT_sb[:, u * P:(u + 1) * P], id_bf[:]
                                )
                                o_sb = opool.tile([P, HS], F32, name="o_sb", tag="o")
                                nc.scalar.activation(
                                    o_sb[:], ps_yt[:], Copy, scale=rec[:]
                                )
                                nc.sync.dma_start(
                                    out[t0:t0 + P, h * HS:(h + 1) * HS], o_sb[:]
                                )
    nc.finalize()
    return nc


_ROPE_PERM = np.concatenate([np.arange(0, DHR, 2), np.arange(1, DHR, 2)])


def _bf(a):
    return np.ascontiguousarray(a).astype(ml_dtypes.bfloat16)


def _prep_inputs(x, freqs_cos, freqs_sin, W_dq, W_uq, W_dkv, W_uk, W_uv, W_qr,
                 W_kr, W_o):
    """Build the 8 per-core input maps (host-side layout prep, all bf16)."""
    x2 = np.asarray(x, np.float32).reshape(T, C)
    xT = x2.T                                        # [C, T]
    wdqT = _bf(np.asarray(W_dq).T.reshape(C, LQ // 4, 512).transpose(1, 0, 2))
    wdkvT = _bf(np.asarray(W_dkv).T.reshape(C, 1, 512).transpose(1, 0, 2))
    wkrT = _bf(np.asarray(W_kr)[_ROPE_PERM, :].T)    # [C, DHR], rope-permuted
    cosT = np.asarray(freqs_cos, np.float32).T       # [32, T]
    sinT = np.asarray(freqs_sin, np.float32).T
    cos2T = _bf(np.concatenate([cosT, cosT], axis=0))    # [64, T]
    sin2T = _bf(np.concatenate([-sinT, sinT], axis=0))
    wuq_full = np.asarray(W_uq).reshape(NLQ, NH * HS)
    wuv = _bf(np.asarray(W_uv).reshape(CCH, P, NLKV))
    W_qr_a = np.asarray(W_qr)
    W_uk_a = np.asarray(W_uk)
    W_o_a = np.asarray(W_o)

    in_maps = []
    for i in range(NCORES):
        h0 = i * HPC
        cols = slice(h0 * HS, (h0 + HPC) * HS)       # 256 output cols
        wqr_rows = np.concatenate(
            [W_qr_a[(h0 + h) * DHR + _ROPE_PERM, :] for h in range(HPC)], axis=0
        )                                            # [HPC*64=128, NLQ]
        in_maps.append({
            "xT_sh": _bf(xT[:, i * TS:(i + 1) * TS]),
            "wdqT": wdqT,
            "wdkvT": wdkvT,
            "wkrT": wkrT,
            "cos2T": cos2T,
            "sin2T": sin2T,
            "wuq": _bf(np.ascontiguousarray(wuq_full[:, cols])
                       .reshape(LQ, P, HPC * HS)),
            "wqrT": _bf(np.ascontiguousarray(wqr_rows.T)
                        .reshape(LQ, P, HPC * DHR)),
            "wukT": _bf(np.ascontiguousarray(
                        W_uk_a[h0 * HS:(h0 + HPC) * HS, :].T)
                        .reshape(LKV, P, HPC * HS)),
            "wuv": wuv,
            "woT": _bf(np.ascontiguousarray(W_o_a[cols, :].T)
                       .reshape(CCH, P, HPC * HS)),
        })
    return in_maps


_NC_CACHE = None


def kernel(**inputs):
    global _NC_CACHE
    in_maps = _prep_inputs(**inputs)
    if _NC_CACHE is None:
        _NC_CACHE = build_nc()
    res = run_bass_kernel_spmd(_NC_CACHE, in_maps, core_ids=list(range(NCORES)))
    outs = [np.asarray(res.results[i]["out"], np.float32) for i in range(NCORES)]
    y = np.concatenate(outs, axis=1).reshape(B, T, C)
    return y


# revision 16
# speedup vs baseline: 1.1199x; 1.0129x over previous
"""MLA-style attention (nn_Attention_7868380086611) on 8 TRN2 NeuronCores.

Strategy
--------
The reference "absorbs" the up-projections (k_eff = Wuq_h @ Wuk_h per head,
v_eff = (W_uv.T @ W_o.T) per-head slices), which is ~4x more FLOPs than the
factored form.  By matmul associativity we instead compute standard per-head
q/k (head dim 128) plus the decoupled-RoPE part, and an effective per-head
v~_h = c_kv @ (W_uv.T @ W_o.T)[:, cols_h], so the [T,T] attention matrix only
ever multiplies 128-wide tensors.

Sharding: head-parallel attention (2 of 16 heads per core) on top of
T-sharded down-projections.  Each core computes c_q/c_kv/k_r for its T/8
token slice (transposed layout, contraction dims on partitions), then one
AllGather (~1 MB/rank, bf16) replicates the tiny latents, and each core runs
the full causal attention for its 2 heads, writing its own 256 output
columns.  All inputs are pre-cast/pre-tiled to bf16 on the host; PSUM
accumulation is fp32.

The same SPMD graph runs on all 8 cores; all rank-dependence is carried by
the per-core input slices.
"""

import math
import sys

import numpy as np

sys.path.insert(0, "/opt/trn_rl_repo")

import ml_dtypes  # noqa: E402

from concourse import bacc, bass, masks, mybir  # noqa: E402
from concourse.bass_utils import run_bass_kernel_spmd  # noqa: E402
from concourse.tile import TileContext  # noqa: E402

B, T, C = 1, 2048, 2048
NH, HS = 16, 128
NLQ, NLKV, DHR = 1536, 512, 64
NCORES = 8
HPC = NH // NCORES          # heads per core = 2
TS = T // NCORES            # 256-token shard for down-projections
P = 128
LQ = NLQ // P               # 12 l-chunks
LKV = NLKV // P             # 4
CCH = C // P                # 16 c-chunks
TJ = T // 512               # 4 t-chunks of 512
SC = T // P                 # 16 s-chunks
SCALE = 1.0 / math.sqrt(HS + DHR)
NEG = -1.0e10

BF = mybir.dt.bfloat16
F32 = mybir.dt.float32
Exp = mybir.ActivationFunctionType.Exp
Copy = mybir.ActivationFunctionType.Copy

GROUP = NLQ + NLKV + DHR    # 2112 rows in the all-gather buffer


def build_nc():
    nc = bacc.Bacc(None, target_bir_lowering=False, num_devices=NCORES)

    xT_sh = nc.declare_dram_parameter("xT_sh", [C, TS], BF, isOutput=False)
    wdqT = nc.declare_dram_parameter("wdqT", [LQ // 4, C, 512], BF, isOutput=False)
    wdkvT = nc.declare_dram_parameter("wdkvT", [1, C, 512], BF, isOutput=False)
    wkrT = nc.declare_dram_parameter("wkrT", [C, DHR], BF, isOutput=False)
    cos2T = nc.declare_dram_parameter("cos2T", [DHR, T], BF, isOutput=False)
    sin2T = nc.declare_dram_parameter("sin2T", [DHR, T], BF, isOutput=False)
    wuq = nc.declare_dram_parameter("wuq", [LQ, P, HPC * HS], BF, isOutput=False)
    wqrT = nc.declare_dram_parameter("wqrT", [LQ, P, HPC * DHR], BF, isOutput=False)
    wukT = nc.declare_dram_parameter("wukT", [LKV, P, HPC * HS], BF, isOutput=False)
    wuv = nc.declare_dram_parameter("wuv", [CCH, P, NLKV], BF, isOutput=False)
    woT = nc.declare_dram_parameter("woT", [CCH, P, HPC * HS], BF, isOutput=False)
    out = nc.declare_dram_parameter("out", [T, HPC * HS], F32, isOutput=True)

    GKV = NLKV + DHR
    cc_in_kv = nc.dram_tensor("cc_in_kv", [GKV, TS], BF)
    cc_out_kv = nc.dram_tensor("cc_out_kv", [NCORES, GKV, TS], BF,
                               addr_space="Shared")
    cc_in_q = nc.dram_tensor("cc_in_q", [NLQ, TS], BF)
    cc_out_q = nc.dram_tensor("cc_out_q", [NCORES, NLQ, TS], BF,
                              addr_space="Shared")

    with TileContext(nc) as tc:
        with (
            tc.tile_pool(name="persist", bufs=1) as persist,
            tc.tile_pool(name="lat", bufs=1) as lat,
            tc.tile_pool(name="proj", bufs=1) as proj,
            tc.tile_pool(name="wts", bufs=1) as wts,
        ):
            # ---- constants ----
            id_bf = persist.tile([P, P], BF)
            masks.make_identity(nc, id_bf[:])
            id_f32 = persist.tile([P, P], F32)
            masks.make_identity(nc, id_f32[:])
            ones_bf = persist.tile([P, 1], BF)
            nc.vector.memset(ones_bf[:], 1.0)
            # 4 additive causal masks [128, 512]: keep (0) iff t - s - 128*m >= 0
            cmask = persist.tile([P, 4 * 512], F32)
            nc.gpsimd.memset(cmask[:], 0.0)
            for m in range(4):
                nc.gpsimd.affine_select(
                    out=cmask[:, m * 512:(m + 1) * 512],
                    in_=cmask[:, m * 512:(m + 1) * 512],
                    compare_op=mybir.AluOpType.is_ge,
                    fill=NEG,
                    base=-m * P,
                    channel_multiplier=-1,
                    pattern=[[1, 512]],
                )
            cos_sb = persist.tile([DHR, T], BF)
            nc.sync.dma_start(cos_sb[:], cos2T[:, :])
            sin_sb = persist.tile([DHR, T], BF)
            nc.sync.dma_start(sin_sb[:], sin2T[:, :])

            # ---- phase 1: c_q^T shard -> AG-q; then c_kv^T/k_r^T -> AG-kv.
            # AG-q goes first: its dependent work (q/qr proj, 74k PE rows) is
            # 2.3x the kv-side's, so the kv collective hides under it.
            with (
                tc.tile_pool(name="p1w", bufs=2) as p1w,
                tc.tile_pool(name="p1ps", bufs=2, space="PSUM") as p1ps,
                tc.tile_pool(name="p1sh", bufs=3) as p1sh,
            ):
                xt = []
                for g in range(4):
                    t = lat.tile([P, 4 * TS], BF, name=f"xt{g}", tag=f"xt{g}")
                    nc.sync.dma_start(
                        t[:].rearrange("p (n u) -> p n u", n=4),
                        xT_sh.ap()
                        .rearrange("(n p) u -> n p u", p=P)[4 * g:4 * (g + 1)]
                        .rearrange("n p u -> p n u"),
                    )
                    xt.append(t)

                def xtile(c):
                    return xt[c // 4][:, (c % 4) * TS:(c % 4 + 1) * TS]

                def down_proj(wparam, group, nsub, bounce, row0):
                    w = p1w.tile([P, CCH * nsub * P], BF, name="p1w_t", tag="p1w_t")
                    nc.sync.dma_start(
                        w[:].rearrange("p (n m) -> p n m", n=CCH),
                        wparam[group].rearrange("(n p) m -> p n m", p=P),
                    )
                    for ls in range(nsub):
                        ps = p1ps.tile([P, TS], F32, name="p1ps_t", tag="p1ps_t")
                        for c in range(CCH):
                            nc.tensor.matmul(
                                ps[:],
                                w[:, c * nsub * P + ls * P:
                                  c * nsub * P + (ls + 1) * P],
                                xtile(c),
                                start=(c == 0),
                                stop=(c == CCH - 1),
                            )
                        sh = p1sh.tile([P, TS], BF, name="p1sh_t", tag="p1sh_t")
                        nc.scalar.copy(sh[:], ps[:])
                        nc.scalar.dma_start(
                            bounce[row0 + ls * P: row0 + (ls + 1) * P, :], sh[:]
                        )

                # c_q (12 l-chunks in 3 groups of 4), then AG-q
                for g in range(LQ // 4):
                    down_proj(wdqT, g, 4, cc_in_q, g * 4 * P)
                nc.gpsimd.collective_compute(
                    "AllGather",
                    mybir.AluOpType.bypass,
                    replica_groups=[list(range(NCORES))],
                    ins=[cc_in_q.ap().opt()],
                    outs=[cc_out_q.ap().opt()],
                )

                # c_kv (4 l-chunks) then k_r, then AG-kv
                down_proj(wdkvT, 0, 4, cc_in_kv, 0)
                wkr_sb = p1w.tile([P, CCH * DHR], BF, name="wkr_sb")
                nc.sync.dma_start(
                    wkr_sb[:].rearrange("p (n m) -> p n m", n=CCH),
                    wkrT.ap().rearrange("(n p) m -> p n m", p=P),
                )
                ps_kr = p1ps.tile([DHR, TS], F32, name="ps_kr", tag="p1ps_t")
                for c in range(CCH):
                    nc.tensor.matmul(
                        ps_kr[:],
                        wkr_sb[:, c * DHR:(c + 1) * DHR],
                        xtile(c),
                        start=(c == 0),
                        stop=(c == CCH - 1),
                    )
                sh_kr = p1sh.tile([DHR, TS], BF, name="sh_kr")
                nc.scalar.copy(sh_kr[:], ps_kr[:])
                nc.scalar.dma_start(cc_in_kv[NLKV:GKV, :], sh_kr[:])

                nc.gpsimd.collective_compute(
                    "AllGather",
                    mybir.AluOpType.bypass,
                    replica_groups=[list(range(NCORES))],
                    ins=[cc_in_kv.ap().opt()],
                    outs=[cc_out_kv.ap().opt()],
                )

            # ---- B = (W_uv.T @ W_o.T)[:, 2-head cols]  (independent of AGs) ----
            b_all = proj.tile([P, LKV * HPC * HS], BF)  # [128, 4*256]
            with (
                tc.tile_pool(name="pbw", bufs=3) as pbw,
                tc.tile_pool(name="pbps", bufs=1, space="PSUM") as pbps,
            ):
                ps_b = [
                    pbps.tile([P, HPC * HS], F32, name=f"ps_b{m}") for m in range(LKV)
                ]
                for c in range(CCH):
                    wuv_t = pbw.tile([P, NLKV], BF, name="wuv_t", tag="wuv_t")
                    nc.sync.dma_start(wuv_t[:], wuv[c])
                    wo_t = pbw.tile([P, HPC * HS], BF, name="wo_t", tag="wo_t")
                    nc.sync.dma_start(wo_t[:], woT[c])
                    for m in range(LKV):
                        nc.tensor.matmul(
                            ps_b[m][:],
                            wuv_t[:, m * P:(m + 1) * P],
                            wo_t[:],
                            start=(c == 0),
                            stop=(c == CCH - 1),
                        )
                for m in range(LKV):
                    nc.vector.tensor_copy(
                        b_all[:, m * HPC * HS:(m + 1) * HPC * HS], ps_b[m][:]
                    )

            # ---- prefetch post-gather projection weights (sync queue, before
            # the collective-gated latent loads) ----
            wuq_all = wts.tile([P, LQ * HPC * HS], BF)
            for l in range(LQ):
                nc.sync.dma_start(
                    wuq_all[:, l * HPC * HS:(l + 1) * HPC * HS], wuq[l]
                )
            wqr_all = wts.tile([P, LQ * HPC * DHR], BF)
            for l in range(LQ):
                nc.sync.dma_start(
                    wqr_all[:, l * HPC * DHR:(l + 1) * HPC * DHR], wqrT[l]
                )
            wuk_all = wts.tile([P, LKV * HPC * HS], BF)
            for l in range(LKV):
                nc.sync.dma_start(
                    wuk_all[:, l * HPC * HS:(l + 1) * HPC * HS], wukT[l]
                )

            # ---- gathered q latent (arrives first) ----
            cq_t = []
            for l in range(LQ):
                t = lat.tile([P, T], BF, name=f"cq{l}", tag=f"cq{l}")
                nc.sync.dma_start(
                    t[:].rearrange("p (g u) -> p g u", g=NCORES),
                    cc_out_q[:, l * P:(l + 1) * P, :].rearrange("g p u -> p g u"),
                )
                cq_t.append(t)

            with tc.tile_pool(name="rtmp", bufs=2) as rtmp:

                def rope(dst, src):
                    # dst = src * [cos;cos] + swap_halves(src) * [-sin;sin]
                    sw = rtmp.tile([DHR, T], BF, name="rsw", tag="rsw")
                    nc.sync.dma_start(sw[0:32, :], src[32:64, :])
                    nc.sync.dma_start(sw[32:64, :], src[0:32, :])
                    ta = rtmp.tile([DHR, T], BF, name="rta", tag="rta")
                    tb = rtmp.tile([DHR, T], BF, name="rtb", tag="rtb")
                    nc.vector.tensor_mul(ta[:], src, cos_sb[:])
                    nc.vector.tensor_mul(tb[:], sw[:], sin_sb[:])
                    nc.vector.tensor_add(dst, ta[:], tb[:])

                qT = proj.tile([P, HPC * T], BF)
                kT = proj.tile([P, HPC * T], BF)
                qr_rope = proj.tile([DHR, HPC * T], BF)
                qr_raw = proj.tile([DHR, HPC * T], BF)
                qr2 = proj.tile([P, T], BF)          # merged 2-head qr, pre-split
                v_sb = proj.tile([P, SC * HPC * HS], BF)
                kr_rope = proj.tile([DHR, T], BF)

                with tc.tile_pool(name="p5ps", bufs=3, space="PSUM") as p5ps:
                    # q_r^T both heads in one matmul (M=128), split after
                    for tj in range(TJ):
                        ps = p5ps.tile([P, 512], F32, name="ps_qr", tag="p5")
                        for l in range(LQ):
                            nc.tensor.matmul(
                                ps[:],
                                wqr_all[:, l * HPC * DHR:(l + 1) * HPC * DHR],
                                cq_t[l][:, tj * 512:(tj + 1) * 512],
                                start=(l == 0),
                                stop=(l == LQ - 1),
                            )
                        nc.vector.tensor_copy(qr2[:, tj * 512:(tj + 1) * 512], ps[:])
                    nc.vector.tensor_copy(qr_raw[:, 0:T], qr2[0:DHR, :])
                    nc.sync.dma_start(qr_raw[:, T:HPC * T], qr2[DHR:P, :])
                    for h in range(HPC):
                        rope(qr_rope[:, h * T:(h + 1) * T],
                             qr_raw[:, h * T:(h + 1) * T])

                    # q^T per head
                    for h in range(HPC):
                        for tj in range(TJ):
                            ps = p5ps.tile([P, 512], F32, name="ps_q", tag="p5")
                            for l in range(LQ):
                                nc.tensor.matmul(
                                    ps[:],
                                    wuq_all[:, l * HPC * HS + h * HS:
                                            l * HPC * HS + (h + 1) * HS],
                                    cq_t[l][:, tj * 512:(tj + 1) * 512],
                                    start=(l == 0),
                                    stop=(l == LQ - 1),
                                )
                            nc.vector.tensor_copy(
                                qT[:, h * T + tj * 512: h * T + (tj + 1) * 512],
                                ps[:],
                            )

                    # gathered kv latents (arrive second)
                    ckv_t = []
                    for l in range(LKV):
                        t = lat.tile([P, T], BF, name=f"ckv{l}", tag=f"ckv{l}")
                        nc.sync.dma_start(
                            t[:].rearrange("p (g u) -> p g u", g=NCORES),
                            cc_out_kv[:, l * P:(l + 1) * P, :].rearrange(
                                "g p u -> p g u"
                            ),
                        )
                        ckv_t.append(t)
                    kr_raw = lat.tile([DHR, T], BF)
                    nc.sync.dma_start(
                        kr_raw[:].rearrange("p (g u) -> p g u", g=NCORES),
                        cc_out_kv[:, NLKV:GKV, :].rearrange("g p u -> p g u"),
                    )
                    rope(kr_rope[:, :], kr_raw[:, :])

                    # k^T per head
                    for h in range(HPC):
                        for sj in range(TJ):
                            ps = p5ps.tile([P, 512], F32, name="ps_k", tag="p5")
                            for l in range(LKV):
                                nc.tensor.matmul(
                                    ps[:],
                                    wuk_all[:, l * HPC * HS + h * HS:
                                            l * HPC * HS + (h + 1) * HS],
                                    ckv_t[l][:, sj * 512:(sj + 1) * 512],
                                    start=(l == 0),
                                    stop=(l == LKV - 1),
                                )
                            nc.vector.tensor_copy(
                                kT[:, h * T + sj * 512: h * T + (sj + 1) * 512],
                                ps[:],
                            )
                    # v~ per s-chunk
                    for sc in range(SC):
                        ps = p5ps.tile([P, HPC * HS], F32, name="ps_v", tag="p5")
                        for l in range(LKV):
                            nc.tensor.matmul(
                                ps[:],
                                ckv_t[l][:, sc * P:(sc + 1) * P],
                                b_all[:, l * HPC * HS:(l + 1) * HPC * HS],
                                start=(l == 0),
                                stop=(l == LKV - 1),
                            )
                        nc.vector.tensor_copy(
                            v_sb[:, sc * HPC * HS:(sc + 1) * HPC * HS], ps[:]
                        )

                # ---- attention (causal, per head, transposed-scores flow) ----
                with (
                    tc.tile_pool(name="pss", bufs=3, space="PSUM") as pss,
                    tc.tile_pool(name="psy", bufs=2, space="PSUM") as psy,
                    tc.tile_pool(name="psd", bufs=1, space="PSUM") as psd,
                    tc.tile_pool(name="psdt", bufs=1, space="PSUM") as psdt,
                    tc.tile_pool(name="psyt", bufs=1, space="PSUM") as psyt,
                    tc.tile_pool(name="atp", bufs=6) as atp,
                    tc.tile_pool(name="spool", bufs=3) as spool,
                    tc.tile_pool(name="opool", bufs=3) as opool,
                ):
                    for h in range(HPC):
                        for tj in range(TJ):
                            nsc = 4 * (tj + 1)
                            ps_y = psy.tile([P, 512], F32, name="ps_y", tag="psy")
                            ps_d = psd.tile([1, 512], F32, name="ps_d", tag="psd")
                            for k in range(nsc):
                                ps_s = pss.tile([P, 512], F32, name="ps_s", tag="pss")
                                nc.tensor.matmul(
                                    ps_s[:],
                                    kT[:, h * T + k * P: h * T + (k + 1) * P],
                                    qT[:, h * T + tj * 512: h * T + (tj + 1) * 512],
                                    start=True,
                                    stop=False,
                                )
                                nc.tensor.matmul(
                                    ps_s[:],
                                    kr_rope[:, k * P:(k + 1) * P],
                                    qr_rope[:, h * T + tj * 512:
                                            h * T + (tj + 1) * 512],
                                    start=False,
                                    stop=True,
                                )
                                m = k - 4 * tj
                                if m >= 0:
                                    nc.vector.tensor_add(
                                        ps_s[:], ps_s[:],
                                        cmask[:, m * 512:(m + 1) * 512],
                                    )
                                at = atp.tile([P, 512], BF, name="at", tag="at")
                                nc.scalar.activation(at[:], ps_s[:], Exp, scale=SCALE)
                                nc.tensor.matmul(
                                    ps_y[:],
                                    v_sb[:, k * HPC * HS + h * HS:
                                         k * HPC * HS + (h + 1) * HS],
                                    at[:],
                                    start=(k == 0),
                                    stop=(k == nsc - 1),
                                )
                                nc.tensor.matmul(
                                    ps_d[:],
                                    ones_bf[:],
                                    at[:],
                                    start=(k == 0),
                                    stop=(k == nsc - 1),
                                )
                            den_sb = spool.tile([1, 512], F32, name="den", tag="den")
                            nc.vector.tensor_copy(den_sb[:], ps_d[:])
                            yT_sb = spool.tile([P, 512], BF, name="yT", tag="yT")
                            nc.scalar.copy(yT_sb[:], ps_y[:])
                            for u in range(4):
                                t0 = tj * 512 + u * P
                                ps_dt = psdt.tile([P, 1], F32, name="ps_dt",
                                                  tag="psdt")
                                nc.tensor.transpose(
                                    ps_dt[:], den_sb[:, u * P:(u + 1) * P],
                                    id_f32[:1, :1],
                                )
                                rec = spool.tile([P, 1], F32, name="rec", tag="rec")
                                nc.vector.reciprocal(rec[:], ps_dt[:])
                                ps_yt = psyt.tile([P, P], BF, name="ps_yt",
                                                  tag="psyt")
                                nc.tensor.transpose(
                                    ps_yt[:], yT_sb[:, u * P:(u + 1) * P], id_bf[:]
                                )
                                o_sb = opool.tile([P, HS], F32, name="o_sb", tag="o")
                                nc.scalar.activation(
                                    o_sb[:], ps_yt[:], Copy, scale=rec[:]
                                )
                                nc.sync.dma_start(
                                    out[t0:t0 + P, h * HS:(h + 1) * HS], o_sb[:]
                                )
    nc.finalize()
    return nc


_ROPE_PERM = np.concatenate([np.arange(0, DHR, 2), np.arange(1, DHR, 2)])


def _bf(a):
    return np.ascontiguousarray(a).astype(ml_dtypes.bfloat16)


def _prep_inputs(x, freqs_cos, freqs_sin, W_dq, W_uq, W_dkv, W_uk, W_uv, W_qr,
                 W_kr, W_o):
    """Build the 8 per-core input maps (host-side layout prep, all bf16)."""
    x2 = np.asarray(x, np.float32).reshape(T, C)
    xT = x2.T                                        # [C, T]
    wdqT = _bf(np.asarray(W_dq).T.reshape(C, LQ // 4, 512).transpose(1, 0, 2))
    wdkvT = _bf(np.asarray(W_dkv).T.reshape(C, 1, 512).transpose(1, 0, 2))
    wkrT = _bf(np.asarray(W_kr)[_ROPE_PERM, :].T)    # [C, DHR], rope-permuted
    cosT = np.asarray(freqs_cos, np.float32).T       # [32, T]
    sinT = np.asarray(freqs_sin, np.float32).T
    cos2T = _bf(np.concatenate([cosT, cosT], axis=0))    # [64, T]
    sin2T = _bf(np.concatenate([-sinT, sinT], axis=0))
    wuq_full = np.asarray(W_uq).reshape(NLQ, NH * HS)
    wuv = _bf(np.asarray(W_uv).reshape(CCH, P, NLKV))
    W_qr_a = np.asarray(W_qr)
    W_uk_a = np.asarray(W_uk)
    W_o_a = np.asarray(W_o)

    in_maps = []
    for i in range(NCORES):
        h0 = i * HPC
        cols = slice(h0 * HS, (h0 + HPC) * HS)       # 256 output cols
        wqr_rows = np.concatenate(
            [W_qr_a[(h0 + h) * DHR + _ROPE_PERM, :] for h in range(HPC)], axis=0
        )                                            # [HPC*64=128, NLQ]
        in_maps.append({
            "xT_sh": _bf(xT[:, i * TS:(i + 1) * TS]),
            "wdqT": wdqT,
            "wdkvT": wdkvT,
            "wkrT": wkrT,
            "cos2T": cos2T,
            "sin2T": sin2T,
            "wuq": _bf(np.ascontiguousarray(wuq_full[:, cols])
                       .reshape(LQ, P, HPC * HS)),
            "wqrT": _bf(np.ascontiguousarray(wqr_rows.T)
                        .reshape(LQ, P, HPC * DHR)),
            "wukT": _bf(np.ascontiguousarray(
                        W_uk_a[h0 * HS:(h0 + HPC) * HS, :].T)
                        .reshape(LKV, P, HPC * HS)),
            "wuv": wuv,
            "woT": _bf(np.ascontiguousarray(W_o_a[cols, :].T)
                       .reshape(CCH, P, HPC * HS)),
        })
    return in_maps


_NC_CACHE = None


def kernel(**inputs):
    global _NC_CACHE
    in_maps = _prep_inputs(**inputs)
    if _NC_CACHE is None:
        _NC_CACHE = build_nc()
    res = run_bass_kernel_spmd(_NC_CACHE, in_maps, core_ids=list(range(NCORES)))
    outs = [np.asarray(res.results[i]["out"], np.float32) for i in range(NCORES)]
    y = np.concatenate(outs, axis=1).reshape(B, T, C)
    return y
